# revision 6
# baseline (speedup 1.0000x reference)
"""Tensor-parallel GQA attention block on 8 TRN2 NeuronCores (Bass/Tile).

Problem: B=1, S=2048, DIM=4096, 32 q heads / 8 kv heads (GQA), head_dim=128,
RoPE, causal softmax, output projection.

Sharding (tensor parallel by head, per the hint): core c of 8 owns q heads
4c..4c+3 and kv head c (GQA groups stay with their q heads). wqkv rows and wo
columns are sharded by head; attention is fully local per core; each core
emits a partial (S, DIM) output (its heads through its wo column slice) and
the partials are summed on the host at unshard time (the "all-reduce after
wo" of the hint, done off-device since full I/O passes through the host
anyway).

Per-core device kernel -- all operands host-pre-transposed so every matmul has
its contraction dim on SBUF partitions; zero on-device transposes:
  qkT = wqkT.T @ xT              (head dims on partitions, seq free)
  v   = xT.T @ wvT               (seq on partitions, head dim free)
  RoPE on qT/kT in transposed layout: host permutes rows into re(0..63)/
    im(64..127); cos/sin arrive as stacked (128, S) tables [cos;cos] and
    [-sin;sin]; 1/sqrt(HD) is folded into wq on the host.
  per head, per 512-wide q chunk (causal: only k tiles <= chunk end):
    S.T[j] = kT_j.T @ qT_chunk   (k positions on partitions => softmax
                                  denominators via a ones-matmul; no P
                                  transpose anywhere)
    P.T[j] = exp(S.T[j] - 12)    (triangular mask added on diagonal tiles;
                                  N trimmed to the causal columns)
    sums  += ones128.T @ P.T[j]  (PSUM-accumulated, rows replicated)
    O.T   += matmul(lhsT=V_j, rhs=P.T[j])
    O.T_norm = O.T * reciprocal_approx(sums)  -> bf16
  out[t, d] = sum_h O.T_h[:, t].T @ woT_h[:, d]

Schedule: quantum-interleaved emission.  Each phase is a generator of PE
"quanta" (~0.6-0.9us of matmuls); a driver round-robins them with ratios so
that during attention (phase B) the exp-dependent ones/PV matmuls always
have an independent projection/output quantum between them and their score
matmul -- the ACT engine's exp latency hides under PE work instead of
stalling it.  Startup congestion is reduced by splitting the first weight
group per k-tile and deferring wo / per-chunk cos-sin DMAs out of the
critical window.  Phase-C PSUM->SBUF eviction alternates ACT/DVE so neither
engine saturates at the tail.

Compute in bf16 with f32 PSUM accumulation; rel l2 error vs the f32
reference is ~8e-3.
"""
import sys

sys.path.insert(0, "/opt/trn_rl_repo")

from contextlib import ExitStack

import numpy as np
import ml_dtypes

import concourse.bass as bass
import concourse.tile as tile
import concourse.mybir as mybir
from concourse import bacc
from concourse.bass_utils import run_bass_kernel_spmd

F32 = mybir.dt.float32
BF16 = mybir.dt.bfloat16
NPBF16 = ml_dtypes.bfloat16

NH, NKV, HD = 32, 8, 128
S, DIM = 2048, 4096
N_CORES = 8
NHL = NH // N_CORES          # q heads per core
PERM = np.concatenate([np.arange(0, 128, 2), np.arange(1, 128, 2)])


def build_attention_kernel(nc, S=2048, DIM=4096, C=12.0):
    NHL = 4          # local q heads
    HD = 128
    CHUNK = 512
    P = 128
    NKT = DIM // P         # k tiles over model dim
    NCH = S // CHUNK       # seq chunks
    QKM = NHL + 1          # m-tiles in qk GEMM (4 q heads + 1 k head)
    NDC = DIM // CHUNK     # output dim chunks

    # ---- DRAM I/O ----
    xT = nc.dram_tensor("xT", (DIM, S), BF16, kind="ExternalInput").ap()
    wqkT = nc.dram_tensor("wqkT", (DIM, QKM * P), BF16, kind="ExternalInput").ap()
    wvT = nc.dram_tensor("wvT", (DIM, HD), BF16, kind="ExternalInput").ap()
    woT = nc.dram_tensor("woT", (NHL * HD, DIM), BF16, kind="ExternalInput").ap()
    # cosX rows 0-63 and 64-127 both hold cos; sinX rows 0-63 hold -sin,
    # rows 64-127 hold +sin (see host prep) -- lets RoPE run as 3 full-width
    # DVE ops with matching base partitions.
    cosT = nc.dram_tensor("cosT", (128, S), F32, kind="ExternalInput").ap()
    sinT = nc.dram_tensor("sinT", (128, S), F32, kind="ExternalInput").ap()
    onesW = nc.dram_tensor("onesW", (P, P), BF16, kind="ExternalInput").ap()
    maskT = nc.dram_tensor("maskT", (P, P), F32, kind="ExternalInput").ap()
    out = nc.dram_tensor("out", (S, DIM), BF16, kind="ExternalOutput").ap()

    with tile.TileContext(nc) as tc, ExitStack() as ctx:
        const = ctx.enter_context(tc.tile_pool(name="const", bufs=1))
        resid = ctx.enter_context(tc.tile_pool(name="resid", bufs=1))
        xpool = ctx.enter_context(tc.tile_pool(name="xp", bufs=8))
        ptpool = ctx.enter_context(tc.tile_pool(name="ptp", bufs=6))
        tmppool = ctx.enter_context(tc.tile_pool(name="tmp", bufs=4))
        obpool = ctx.enter_context(tc.tile_pool(name="obp", bufs=8))
        psum = ctx.enter_context(tc.tile_pool(name="psum", bufs=8, space="PSUM"))

        # ---- weights.  Group 0 is split per k-tile so the very first
        # matmul's weights land in ~0.5us; later groups stream 4 k-tiles per
        # descriptor.  wo (4MB) and the cos/sin tables (2MB) are NOT loaded
        # here -- they'd compete with x / wqk for HBM bandwidth during the
        # startup crunch; their DMAs are emitted later, right before their
        # consumers' phase, via load_wo()/the per-chunk cos-sin slices. ----
        NKG = NKT // 4
        wqk_g = [const.tile([P, 4, QKM * P], BF16, tag=f"wqkg{g}", name=f"wqkg{g}")
                 for g in range(NKG)]
        wv_g = [const.tile([P, 4, HD], BF16, tag=f"wvg{g}", name=f"wvg{g}")
                for g in range(NKG)]
        for kt in range(4):    # group 0: one DMA per k-tile
            nc.gpsimd.dma_start(
                wqk_g[0][:, kt], wqkT[kt * P:(kt + 1) * P, :])
            nc.gpsimd.dma_start(
                wv_g[0][:, kt], wvT[kt * P:(kt + 1) * P, :])
        for g in range(1, NKG):
            nc.gpsimd.dma_start(
                wqk_g[g][:],
                wqkT[g * 4 * P:(g + 1) * 4 * P, :].rearrange(
                    "(kt p) m -> p kt m", p=P))
            nc.gpsimd.dma_start(
                wv_g[g][:],
                wvT[g * 4 * P:(g + 1) * 4 * P, :].rearrange(
                    "(kt p) m -> p kt m", p=P))
        wqk_sb = [wqk_g[k // 4][:, k % 4] for k in range(NKT)]
        wv_sb = [wv_g[k // 4][:, k % 4] for k in range(NKT)]
        ones_sb = const.tile([P, P], BF16, tag="ones", name="ones")
        nc.gpsimd.dma_start(ones_sb[:], onesW[:])
        mask_sb = const.tile([P, P], F32, tag="mask", name="mask")
        nc.gpsimd.dma_start(mask_sb[:], maskT[:])
        negC = const.tile([P, 1], F32, tag="negC", name="negC")
        nc.any.memset(negC[:], -C)
        cos_sb = const.tile([P, S], F32, tag="cos", name="cos")
        sin_sb = const.tile([P, S], F32, tag="sin", name="sin")
        wo_sb = const.tile([P, NHL, DIM], BF16, tag="wo", name="wo")

        cs_loaded = [False] * NCH

        def load_cs(ch):
            """cos/sin columns for chunk ch -- emitted at the start of the
            chunk's first A half so the slice has a full phase to arrive."""
            if cs_loaded[ch]:
                return
            cs_loaded[ch] = True
            sl = slice(ch * CHUNK, (ch + 1) * CHUNK)
            nc.gpsimd.dma_start(cos_sb[:, sl], cosT[:, sl])
            nc.gpsimd.dma_start(sin_sb[:, sl], sinT[:, sl])

        load_cs(0)
        load_cs(1)

        def load_wo(half):
            sl = slice(half * (DIM // 2), (half + 1) * (DIM // 2))
            nc.gpsimd.dma_start(
                wo_sb[:, :, sl],
                woT[:, sl].rearrange("(h p) n -> p h n", p=P))

        # resident activations (per chunk tiles for fine-grained deps)
        q_sb = [[resid.tile([P, CHUNK], BF16, tag=f"q{h}_{ch}", name=f"q{h}_{ch}")
                 for ch in range(NCH)] for h in range(NHL)]
        k_sb = [resid.tile([P, CHUNK], BF16, tag=f"k{ch}", name=f"k{ch}")
                for ch in range(NCH)]
        v_sb = [resid.tile([P, CHUNK], BF16, tag=f"v{ch}", name=f"v{ch}")
                for ch in range(NCH)]
        ot_sb = [[resid.tile([P, CHUNK], BF16, tag=f"ot{h}_{ch}", name=f"ot{h}_{ch}")
                  for ch in range(NCH)] for h in range(NHL)]

        def rope_hc(ps, raw_sw, out_tile, hc):
            """ps: (128, CHUNK//2) f32 PSUM [re; im]; raw_sw: bf16 SBUF with
            halves swapped [im; re] (produced by two ACT copies).
            out = ps*cosX + raw_sw*sinX with cosX = [cos; cos],
            sinX = [-sin; +sin]."""
            HC2 = CHUNK // 2
            cos = cos_sb[:, hc * HC2:(hc + 1) * HC2]
            sin = sin_sb[:, hc * HC2:(hc + 1) * HC2]
            t1 = tmppool.tile([P, HC2], F32, tag="t1", name="t1", bufs=3)
            t2 = tmppool.tile([P, HC2], F32, tag="t2", name="t2", bufs=3)
            nc.vector.tensor_mul(t1[:], ps[:], cos)
            nc.vector.tensor_mul(t2[:], raw_sw[:], sin)
            nc.vector.tensor_add(out_tile[:], t1[:], t2[:])

        HC = CHUNK // 2      # 256-wide half chunks: the qk PSUM footprint
        # is 3 banks (two heads packed per bank) + 1 shared V bank.
        vbank = [None]

        def gen_a(hc):
            """qkv projection + RoPE for seq half-chunk hc (generator: one
            quantum per k-tile, then the rope epilogue)."""
            ch, half = hc // 2, hc % 2
            load_cs(ch)
            qk_bank = [psum.tile([P, CHUNK], F32, tag="ps", name="ps")
                       for _ in range((QKM + 1) // 2)]
            if half == 0:
                vbank[0] = psum.tile([P, CHUNK], F32, tag="ps", name="ps")
            ps_v = vbank[0]

            def qk_slice(m):
                return qk_bank[m // 2][:, (m % 2) * HC:(m % 2 + 1) * HC]

            def fetch_x(k):
                xt = xpool.tile([P, HC], BF16, tag="xt", name="xt")
                nc.sync.dma_start(
                    xt[:], xT[k * P:(k + 1) * P, hc * HC:(hc + 1) * HC])
                return xt

            # 2-deep explicit prefetch: with interleaved emission the k-step
            # quanta are spread out, so the DMA must be issued ahead of its
            # consuming quantum rather than relying on back-to-back emission.
            xts = [fetch_x(0), fetch_x(1)]
            for k in range(NKT):
                xt = xts[k % 2]
                if k + 2 < NKT:
                    xts[k % 2] = fetch_x(k + 2)
                for m in range(QKM):
                    nc.tensor.matmul(
                        qk_slice(m), wqk_sb[k][:, m * P:(m + 1) * P], xt[:],
                        start=(k == 0 and m % 2 == 0),
                        stop=(k == NKT - 1 and (m % 2 == 1 or m == QKM - 1)),
                        skip_group_check=True)
                for t in range(2):
                    nc.tensor.matmul(
                        ps_v[:, (2 * half + t) * P:(2 * half + t + 1) * P],
                        xt[:, t * P:(t + 1) * P], wv_sb[k][:],
                        start=(half == 0 and k == 0 and t == 0),
                        stop=(half == 1 and k == NKT - 1 and t == 1),
                        skip_group_check=True)
                yield
            if half == 1:
                nc.scalar.copy(v_sb[ch][:], ps_v[:])
            rawsw = [tmppool.tile([P, HC], BF16, tag=f"qksw{m}", name=f"qksw{m}", bufs=2)
                     for m in range(QKM)]
            order = [NHL] + list(range(NHL))     # k tile first
            for m in order:
                nc.scalar.copy(rawsw[m][0:64, :], qk_slice(m)[64:128, :])
                nc.scalar.copy(rawsw[m][64:128, :], qk_slice(m)[0:64, :])
            yield
            for m in order:
                out_tile = k_sb[ch] if m == NHL else q_sb[m][ch]
                rope_hc(qk_slice(m), rawsw[m],
                        out_tile[:, half * HC:(half + 1) * HC], hc)
                yield

        def gen_b(ch, lookahead=False):
            """attention for all local heads, q chunk ch (causal).  One
            quantum per (head, k-tile) step; the driver inserts an
            independent PE quantum in each gap so exp never stalls the PE.
            With lookahead=True the j+1 score matmul is also emitted before
            the exp-dependent sums/PV of j (used when no filler phase is
            available)."""
            njt = 4 * ch + 4

            def score(h, j):
                o = j - 4 * ch          # >=0: diagonal region, trim N
                lo = max(o, 0) * P      # first valid q column
                ps_st = psum.tile([P, CHUNK], F32, tag="ps", name="ps")
                nc.tensor.matmul(
                    ps_st[:, lo:], k_sb[j // 4][:, (j % 4) * P:(j % 4 + 1) * P],
                    q_sb[h][ch][:, lo:], start=True, stop=True,
                    skip_group_check=True)
                pt = ptpool.tile([P, CHUNK], BF16, tag="pt", name="pt")
                if o >= 0:  # mask the diagonal 128x128 block
                    nc.vector.tensor_add(
                        ps_st[:, o * P:(o + 1) * P],
                        ps_st[:, o * P:(o + 1) * P], mask_sb[:])
                nc.scalar.activation(
                    pt[:, lo:], ps_st[:, lo:],
                    mybir.ActivationFunctionType.Exp, bias=negC[:])
                return pt, lo

            for h in range(NHL):
                ps_sum = psum.tile([P, CHUNK], F32, tag="ps", name="ps")
                ps_ot = psum.tile([P, CHUNK], F32, tag="ps", name="ps")
                nxt = score(h, 0)
                for j in range(njt):
                    pt, lo = nxt
                    if lookahead and j + 1 < njt:
                        nxt = score(h, j + 1)
                    yield               # filler slot: exp(pt_j) runs here
                    if not lookahead and j + 1 < njt:
                        nxt = score(h, j + 1)
                    nc.tensor.matmul(ps_sum[:, lo:], ones_sb[:], pt[:, lo:],
                                     start=(j == 0), stop=(j == njt - 1),
                                     skip_group_check=True)
                    nc.tensor.matmul(
                        ps_ot[:, lo:], v_sb[j // 4][:, (j % 4) * P:(j % 4 + 1) * P],
                        pt[:, lo:], start=(j == 0), stop=(j == njt - 1),
                        skip_group_check=True)
                recip = tmppool.tile([P, CHUNK], F32, tag="recip", name="recip", bufs=2)
                nc.vector.reciprocal_approx_fast(out=recip[:], in_=ps_sum[:])
                nc.vector.tensor_mul(ot_sb[h][ch][:], ps_ot[:], recip[:])
                yield

        def gen_c(ch, dlo=0, dhi=None):
            """output projection for the 4 seq tiles of chunk ch, output dim
            chunks dlo..dhi (generator: one quantum per (t, d) tile).
            PSUM->SBUF eviction alternates ACT/DVE."""
            if dhi is None:
                dhi = NDC
            for tq in range(4):
                t = 4 * ch + tq
                for d in range(dlo, dhi):
                    ps_o = psum.tile([P, CHUNK], F32, tag="ps", name="ps")
                    for h in range(NHL):
                        nc.tensor.matmul(
                            ps_o[:], ot_sb[h][ch][:, tq * P:(tq + 1) * P],
                            wo_sb[:, h, d * CHUNK:(d + 1) * CHUNK],
                            start=(h == 0), stop=(h == NHL - 1),
                            skip_group_check=True)
                    ob = obpool.tile([P, CHUNK], BF16, tag="ob", name="ob")
                    if d % 2 == 0:
                        nc.vector.tensor_scalar_mul(ob[:], ps_o[:], 1.0)
                    else:
                        nc.scalar.copy(ob[:], ps_o[:])
                    # out DMAs all ride the gpsimd (SWDGE) queue: putting
                    # them on sync would head-of-line block the interleaved
                    # A-phase x fetches behind the ob-copy dependency.
                    nc.gpsimd.dma_start(
                        out[t * P:(t + 1) * P, d * CHUNK:(d + 1) * CHUNK], ob[:])
                    yield

        def run(gen):
            for _ in gen:
                pass

        def interleave(*pairs):
            """pairs: (generator, weight).  Weighted fair queueing at
            single-quantum granularity: each step emits one quantum from the
            generator with the highest accumulated credit, so any ratio
            interleaves smoothly instead of in bursts."""
            state = [[g, float(w), 0.0] for g, w in pairs]
            while state:
                tot = sum(st[1] for st in state)
                for st in state:
                    st[2] += st[1] / tot
                st = max(state, key=lambda s: s[2])
                st[2] -= 1.0
                if next(st[0], "__done__") == "__done__":
                    state.remove(st)

        # ---- schedule ----
        # A(hc) covers chunk hc//2; B(b) needs chunks <= b roped and v'd;
        # C(b) needs all of B(b).  Fillers keep exp off the PE critical path.
        run(gen_a(0))
        run(gen_a(1))
        load_wo(0)
        load_wo(1)
        interleave((gen_b(0), 1), (gen_a(2), 2))
        interleave((gen_c(0), 1), (gen_a(3), 1))
        interleave((gen_b(1), 1), (gen_a(4), 1))
        interleave((gen_c(1), 1), (gen_a(5), 1))
        interleave((gen_b(2), 3), (gen_a(6), 2))
        interleave((gen_c(2, 0, 4), 1), (gen_a(7), 2))
        interleave((gen_b(3, lookahead=True), 4), (gen_c(2, 4, 8), 1))
        run(gen_c(3))

    return nc


def _make_in_maps(x, freqs_cis, wqkv, wo):
    scale = np.float32(1.0 / np.sqrt(HD))
    xT = np.ascontiguousarray(np.asarray(x)[0].T).astype(NPBF16)
    cos = freqs_cis[:, :, 0].T.astype(np.float32)        # (64, S)
    sin = freqs_cis[:, :, 1].T.astype(np.float32)
    cosT = np.ascontiguousarray(np.concatenate([cos, cos], 0))   # (128, S)
    sinT = np.ascontiguousarray(np.concatenate([-sin, sin], 0))
    ones = np.ones((128, 128), NPBF16)
    kp = np.arange(128)[:, None]
    qp = np.arange(128)[None, :]
    maskT = np.where(kp <= qp, 0.0, -1e30).astype(np.float32)

    in_maps = []
    for c in range(N_CORES):
        rows = [wqkv[128 * (NHL * c + h) + PERM] * scale for h in range(NHL)]
        rows.append(wqkv[NH * HD + 128 * c + PERM])
        wqkT = np.ascontiguousarray(np.concatenate(rows, 0).T).astype(NPBF16)
        wvT = np.ascontiguousarray(
            wqkv[(NH + NKV) * HD + 128 * c:(NH + NKV) * HD + 128 * (c + 1)].T
        ).astype(NPBF16)
        woT = np.ascontiguousarray(
            wo[:, 128 * NHL * c:128 * NHL * (c + 1)].T).astype(NPBF16)
        in_maps.append({
            "xT": xT, "wqkT": wqkT, "wvT": wvT, "woT": woT,
            "cosT": cosT, "sinT": sinT, "onesW": ones, "maskT": maskT,
        })
    return in_maps


def kernel(x, freqs_cis, wqkv, wo):
    x = np.asarray(x, dtype=np.float32)
    freqs_cis = np.asarray(freqs_cis, dtype=np.float32)
    wqkv = np.asarray(wqkv, dtype=np.float32)
    wo = np.asarray(wo, dtype=np.float32)

    in_maps = _make_in_maps(x, freqs_cis, wqkv, wo)
    nc = bacc.Bacc("TRN2", target_bir_lowering=False, debug=False,
                   num_devices=N_CORES)
    build_attention_kernel(nc, S=S, DIM=DIM)
    nc.compile()
    res = run_bass_kernel_spmd(nc, in_maps, core_ids=list(range(N_CORES)))

    acc = np.zeros((S, DIM), np.float32)
    for r in res.results:
        acc += np.asarray(r["out"]).astype(np.float32)
    return acc[None]


# revision 9
# speedup vs baseline: 1.0364x; 1.0364x over previous
"""Tensor-parallel GQA attention block on 8 TRN2 NeuronCores (Bass/Tile).

Problem: B=1, S=2048, DIM=4096, 32 q heads / 8 kv heads (GQA), head_dim=128,
RoPE, causal softmax, output projection.

Sharding (tensor parallel by head, per the hint): core c of 8 owns q heads
4c..4c+3 and kv head c (GQA groups stay with their q heads). wqkv rows and wo
columns are sharded by head; attention is fully local per core; each core
emits a partial (S, DIM) output (its heads through its wo column slice) and
the partials are summed on the host at unshard time (the "all-reduce after
wo" of the hint, done off-device since full I/O passes through the host
anyway).

Per-core device kernel -- all operands host-pre-transposed so every matmul has
its contraction dim on SBUF partitions; zero on-device transposes:
  qkT = wqkT.T @ xT              (head dims on partitions, seq free)
  v   = xT.T @ wvT               (seq on partitions, head dim free)
  RoPE on qT/kT in transposed layout: host permutes rows into re(0..63)/
    im(64..127); cos/sin arrive as stacked (128, S) tables [cos;cos] and
    [-sin;sin]; 1/sqrt(HD) is folded into wq on the host.
  per head, per 512-wide q chunk (causal: only k tiles <= chunk end):
    S.T[j] = kT_j.T @ qT_chunk   (k positions on partitions => softmax
                                  denominators via a ones-matmul; no P
                                  transpose anywhere)
    P.T[j] = exp(S.T[j] - 12)    (triangular mask added on diagonal tiles;
                                  N trimmed to the causal columns)
    sums  += ones128.T @ P.T[j]  (PSUM-accumulated, rows replicated)
    O.T   += matmul(lhsT=V_j, rhs=P.T[j])
    O.T_norm = O.T * reciprocal_approx(sums)  -> bf16
  out[t, d] = sum_h O.T_h[:, t].T @ woT_h[:, d]

Schedule: quantum-interleaved emission.  Each phase is a generator of PE
"quanta" (~0.6-0.9us of matmuls); a weighted-fair driver mixes them so that
during attention the exp-dependent ones/PV matmuls always have independent
projection/output quanta between them and their score matmul -- the ACT
engine's exp latency hides under PE work instead of stalling it.  Each B
phase's first j-steps are pre-started inside the previous C interleave
(shared generator) so their counting-semaphore thresholds exclude C's
final PSUM evictions.  Weights stream as 9 staggered group DMAs of a
combined wqkv tensor (SWDGE descriptor generation is ~0.7us per DMA, so
descriptor count, not just bytes, gates startup); wo and per-chunk cos/sin
slices load outside the startup window.  PSUM->SBUF evictions alternate
ACT/DVE and out-DMAs rotate across engine queues (4-way at the tail).

Compute in bf16 with f32 PSUM accumulation; rel l2 error vs the f32
reference is ~8e-3.
"""
import sys

sys.path.insert(0, "/opt/trn_rl_repo")

from contextlib import ExitStack

import numpy as np
import ml_dtypes

import concourse.bass as bass
import concourse.tile as tile
import concourse.mybir as mybir
from concourse import bacc
from concourse.bass_utils import run_bass_kernel_spmd

F32 = mybir.dt.float32
BF16 = mybir.dt.bfloat16
NPBF16 = ml_dtypes.bfloat16

NH, NKV, HD = 32, 8, 128
S, DIM = 2048, 4096
N_CORES = 8
NHL = NH // N_CORES          # q heads per core
PERM = np.concatenate([np.arange(0, 128, 2), np.arange(1, 128, 2)])
WGROUPS = [2, 2, 4, 4, 4, 4, 4, 4, 4]   # k-tiles per weight-group DMA


def build_attention_kernel(nc, S=2048, DIM=4096, C=12.0):
    NHL = 4          # local q heads
    HD = 128
    CHUNK = 512
    P = 128
    NKT = DIM // P         # k tiles over model dim
    NCH = S // CHUNK       # seq chunks
    QKM = NHL + 1          # m-tiles in qk GEMM (4 q heads + 1 k head)
    NDC = DIM // CHUNK     # output dim chunks
    WM = QKM * P + HD      # combined wqkv row width (640 qk + 128 v)

    # ---- DRAM I/O ----
    xT = nc.dram_tensor("xT", (DIM, S), BF16, kind="ExternalInput").ap()
    # combined qkv weights: columns 0..639 = wqkT (4 q heads + 1 k head),
    # 640..767 = wvT -- one DMA per k-tile group covers both.
    wqkvT = nc.dram_tensor("wqkvT", (DIM, WM), BF16, kind="ExternalInput").ap()
    woT = nc.dram_tensor("woT", (NHL * HD, DIM), BF16, kind="ExternalInput").ap()
    # csT[:, ch, 0, :] = cos columns of chunk ch ([cos;cos] stacked rows),
    # csT[:, ch, 1, :] = sin columns ([-sin;+sin]) -- one DMA per chunk.
    csT = nc.dram_tensor("csT", (128, NCH, 2, CHUNK), F32,
                         kind="ExternalInput").ap()
    onesW = nc.dram_tensor("onesW", (P, P), BF16, kind="ExternalInput").ap()
    maskT = nc.dram_tensor("maskT", (P, P), F32, kind="ExternalInput").ap()
    out = nc.dram_tensor("out", (S, DIM), BF16, kind="ExternalOutput").ap()

    with tile.TileContext(nc) as tc, ExitStack() as ctx:
        const = ctx.enter_context(tc.tile_pool(name="const", bufs=1))
        resid = ctx.enter_context(tc.tile_pool(name="resid", bufs=1))
        xpool = ctx.enter_context(tc.tile_pool(name="xp", bufs=8))
        ptpool = ctx.enter_context(tc.tile_pool(name="ptp", bufs=6))
        tmppool = ctx.enter_context(tc.tile_pool(name="tmp", bufs=4))
        obpool = ctx.enter_context(tc.tile_pool(name="obp", bufs=8))
        psum = ctx.enter_context(tc.tile_pool(name="psum", bufs=8, space="PSUM"))

        # ---- weights: 9 staggered group DMAs (small groups first so the
        # first matmul starts in ~2us, larger ones amortize the per-DMA
        # SWDGE descriptor cost while the wire streams). ----
        w_g = []
        k0 = 0
        for gi, gn in enumerate(WGROUPS):
            g = const.tile([P, gn, WM], BF16, tag=f"wg{gi}", name=f"wg{gi}")
            nc.gpsimd.dma_start(
                g[:], wqkvT[k0 * P:(k0 + gn) * P, :].rearrange(
                    "(kt p) m -> p kt m", p=P))
            w_g.append((g, k0))
            k0 += gn
        ktile = []
        for (g, k0), gn in zip(w_g, WGROUPS):
            for i in range(gn):
                ktile.append(g[:, i])
        wqk_sb = [ktile[k][:, 0:QKM * P] for k in range(NKT)]
        wv_sb = [ktile[k][:, QKM * P:WM] for k in range(NKT)]
        ones_sb = const.tile([P, P], BF16, tag="ones", name="ones")
        nc.gpsimd.dma_start(ones_sb[:], onesW[:])
        mask_sb = const.tile([P, P], F32, tag="mask", name="mask")
        nc.gpsimd.dma_start(mask_sb[:], maskT[:])
        negC = const.tile([P, 1], F32, tag="negC", name="negC")
        nc.any.memset(negC[:], -C)
        cs_sb = const.tile([P, NCH, 2, CHUNK], F32, tag="cs", name="cs")
        wo_sb = const.tile([P, NHL, DIM], BF16, tag="wo", name="wo")

        cs_loaded = [False] * NCH

        def load_cs(ch):
            """cos/sin columns for chunk ch -- emitted at the start of the
            chunk's first A half so the slice has a full phase to arrive."""
            if cs_loaded[ch]:
                return
            cs_loaded[ch] = True
            nc.gpsimd.dma_start(cs_sb[:, ch], csT[:, ch])

        load_cs(0)
        load_cs(1)

        def load_wo(half):
            sl = slice(half * (DIM // 2), (half + 1) * (DIM // 2))
            nc.gpsimd.dma_start(
                wo_sb[:, :, sl],
                woT[:, sl].rearrange("(h p) n -> p h n", p=P))

        # resident activations (per chunk tiles for fine-grained deps)
        q_sb = [[resid.tile([P, CHUNK], BF16, tag=f"q{h}_{ch}", name=f"q{h}_{ch}")
                 for ch in range(NCH)] for h in range(NHL)]
        k_sb = [resid.tile([P, CHUNK], BF16, tag=f"k{ch}", name=f"k{ch}")
                for ch in range(NCH)]
        v_sb = [resid.tile([P, CHUNK], BF16, tag=f"v{ch}", name=f"v{ch}")
                for ch in range(NCH)]
        ot_sb = [[resid.tile([P, CHUNK], BF16, tag=f"ot{h}_{ch}", name=f"ot{h}_{ch}")
                  for ch in range(NCH)] for h in range(NHL)]

        def rope_hc(ps, raw_sw, out_tile, hc):
            """ps: (128, CHUNK//2) f32 PSUM [re; im]; raw_sw: bf16 SBUF with
            halves swapped [im; re] (produced by two ACT copies).
            out = ps*cosX + raw_sw*sinX with cosX = [cos; cos],
            sinX = [-sin; +sin]."""
            ch, half = hc // 2, hc % 2
            HC2 = CHUNK // 2
            cos = cs_sb[:, ch, 0, half * HC2:(half + 1) * HC2]
            sin = cs_sb[:, ch, 1, half * HC2:(half + 1) * HC2]
            t1 = tmppool.tile([P, HC2], F32, tag="t1", name="t1", bufs=3)
            t2 = tmppool.tile([P, HC2], F32, tag="t2", name="t2", bufs=3)
            nc.vector.tensor_mul(t1[:], ps[:], cos)
            nc.vector.tensor_mul(t2[:], raw_sw[:], sin)
            nc.vector.tensor_add(out_tile[:], t1[:], t2[:])

        HC = CHUNK // 2      # 256-wide half chunks: the qk PSUM footprint
        # is 3 banks (two heads packed per bank) + 1 shared V bank.
        vbank = [None]

        def gen_a(hc):
            """qkv projection + RoPE for seq half-chunk hc (generator: one
            quantum per k-tile, then the rope epilogue)."""
            ch, half = hc // 2, hc % 2
            load_cs(ch)
            qk_bank = [psum.tile([P, CHUNK], F32, tag="ps", name="ps")
                       for _ in range((QKM + 1) // 2)]
            if half == 0:
                vbank[0] = psum.tile([P, CHUNK], F32, tag="ps", name="ps")
            ps_v = vbank[0]

            def qk_slice(m):
                return qk_bank[m // 2][:, (m % 2) * HC:(m % 2 + 1) * HC]

            def fetch_x(k):
                xt = xpool.tile([P, HC], BF16, tag="xt", name="xt")
                nc.sync.dma_start(
                    xt[:], xT[k * P:(k + 1) * P, hc * HC:(hc + 1) * HC])
                return xt

            # 4-deep explicit prefetch: with interleaved emission the k-step
            # quanta are spread out, so DMAs must be issued well ahead of
            # their consuming quantum.
            PF = 4
            xts = [fetch_x(k) for k in range(PF)]
            for k in range(NKT):
                xt = xts[k % PF]
                if k + PF < NKT:
                    xts[k % PF] = fetch_x(k + PF)
                for m in range(QKM):
                    nc.tensor.matmul(
                        qk_slice(m), wqk_sb[k][:, m * P:(m + 1) * P], xt[:],
                        start=(k == 0 and m % 2 == 0),
                        stop=(k == NKT - 1 and (m % 2 == 1 or m == QKM - 1)),
                        skip_group_check=True)
                for t in range(2):
                    nc.tensor.matmul(
                        ps_v[:, (2 * half + t) * P:(2 * half + t + 1) * P],
                        xt[:, t * P:(t + 1) * P], wv_sb[k][:],
                        start=(half == 0 and k == 0 and t == 0),
                        stop=(half == 1 and k == NKT - 1 and t == 1),
                        skip_group_check=True)
                yield
            if half == 1:
                nc.scalar.copy(v_sb[ch][:], ps_v[:])
            rawsw = [tmppool.tile([P, HC], BF16, tag=f"qksw{m}", name=f"qksw{m}", bufs=2)
                     for m in range(QKM)]
            order = [NHL] + list(range(NHL))     # k tile first
            for m in order:
                nc.scalar.copy(rawsw[m][0:64, :], qk_slice(m)[64:128, :])
                nc.scalar.copy(rawsw[m][64:128, :], qk_slice(m)[0:64, :])
            yield
            for m in order:
                out_tile = k_sb[ch] if m == NHL else q_sb[m][ch]
                rope_hc(qk_slice(m), rawsw[m],
                        out_tile[:, half * HC:(half + 1) * HC], hc)
                yield

        def gen_b(ch, lookahead=False):
            """attention for all local heads, q chunk ch (causal).  One
            quantum per (head, k-tile) step; the driver inserts an
            independent PE quantum in each gap so exp never stalls the PE.
            With lookahead=True the j+1 score matmul is also emitted before
            the exp-dependent sums/PV of j (used when little filler is
            available)."""
            njt = 4 * ch + 4

            def score(h, j):
                o = j - 4 * ch          # >=0: diagonal region, trim N
                lo = max(o, 0) * P      # first valid q column
                ps_st = psum.tile([P, CHUNK], F32, tag="ps", name="ps")
                nc.tensor.matmul(
                    ps_st[:, lo:], k_sb[j // 4][:, (j % 4) * P:(j % 4 + 1) * P],
                    q_sb[h][ch][:, lo:], start=True, stop=True,
                    skip_group_check=True)
                pt = ptpool.tile([P, CHUNK], BF16, tag="pt", name="pt")
                if o >= 0:  # mask the diagonal 128x128 block
                    nc.vector.tensor_add(
                        ps_st[:, o * P:(o + 1) * P],
                        ps_st[:, o * P:(o + 1) * P], mask_sb[:])
                nc.scalar.activation(
                    pt[:, lo:], ps_st[:, lo:],
                    mybir.ActivationFunctionType.Exp, bias=negC[:])
                return pt, lo

            for h in range(NHL):
                ps_sum = psum.tile([P, CHUNK], F32, tag="ps", name="ps")
                ps_ot = psum.tile([P, CHUNK], F32, tag="ps", name="ps")
                nxt = score(h, 0)
                for j in range(njt):
                    pt, lo = nxt
                    if lookahead and j + 1 < njt:
                        nxt = score(h, j + 1)
                    yield               # filler slot: exp(pt_j) runs here
                    if not lookahead and j + 1 < njt:
                        nxt = score(h, j + 1)
                    nc.tensor.matmul(ps_sum[:, lo:], ones_sb[:], pt[:, lo:],
                                     start=(j == 0), stop=(j == njt - 1),
                                     skip_group_check=True)
                    nc.tensor.matmul(
                        ps_ot[:, lo:], v_sb[j // 4][:, (j % 4) * P:(j % 4 + 1) * P],
                        pt[:, lo:], start=(j == 0), stop=(j == njt - 1),
                        skip_group_check=True)
                recip = tmppool.tile([P, CHUNK], F32, tag="recip", name="recip", bufs=2)
                nc.vector.reciprocal_approx_fast(out=recip[:], in_=ps_sum[:])
                nc.vector.tensor_mul(ot_sb[h][ch][:], ps_ot[:], recip[:])
                yield

        def gen_c(ch, dlo=0, dhi=None, tail=False):
            """output projection for the 4 seq tiles of chunk ch, output dim
            chunks dlo..dhi (generator: one quantum per (t, d) tile).
            PSUM->SBUF eviction alternates ACT/DVE; out-DMAs rotate across
            engine queues (4-way incl. sync when tail=True -- no x fetches
            follow that could be head-of-line blocked)."""
            if dhi is None:
                dhi = NDC
            qs = ([nc.sync, nc.gpsimd, nc.scalar] if tail
                  else [nc.gpsimd, nc.scalar])
            qi = 0
            for tq in range(4):
                t = 4 * ch + tq
                for d in range(dlo, dhi):
                    ps_o = psum.tile([P, CHUNK], F32, tag="ps", name="ps")
                    for h in range(NHL):
                        nc.tensor.matmul(
                            ps_o[:], ot_sb[h][ch][:, tq * P:(tq + 1) * P],
                            wo_sb[:, h, d * CHUNK:(d + 1) * CHUNK],
                            start=(h == 0), stop=(h == NHL - 1),
                            skip_group_check=True)
                    ob = obpool.tile([P, CHUNK], BF16, tag="ob", name="ob")
                    if d % 2 == 0:
                        nc.vector.tensor_scalar_mul(ob[:], ps_o[:], 1.0)
                    else:
                        nc.scalar.copy(ob[:], ps_o[:])
                    qs[qi % len(qs)].dma_start(
                        out[t * P:(t + 1) * P, d * CHUNK:(d + 1) * CHUNK], ob[:])
                    qi += 1
                    yield

        def run(gen):
            for _ in gen:
                pass

        def take(gen, n):
            """yield up to n quanta from a shared generator."""
            for _ in range(n):
                if next(gen, "__done__") == "__done__":
                    return
                yield

        def chain(*gens):
            for g in gens:
                yield from g

        def interleave(*pairs):
            """pairs: (generator, weight).  Weighted fair queueing at
            single-quantum granularity: each step emits one quantum from the
            generator with the highest accumulated credit, so any ratio
            interleaves smoothly instead of in bursts."""
            state = [[g, float(w), 0.0] for g, w in pairs]
            while state:
                tot = sum(st[1] for st in state)
                for st in state:
                    st[2] += st[1] / tot
                st = max(state, key=lambda s: s[2])
                st[2] -= 1.0
                if next(st[0], "__done__") == "__done__":
                    state.remove(st)

        # ---- schedule ----
        # A(hc) covers chunk hc//2; B(b) needs chunks <= b roped and v'd;
        # C(b) needs all of B(b).  Fillers keep exp off the PE critical
        # path.  Each B's head is pre-started inside the previous C
        # interleave (after the A epilogue that ropes its chunk) so its
        # counting-semaphore thresholds exclude that C's final evictions.
        b1 = gen_b(1)
        b2 = gen_b(2)
        b3 = gen_b(3, lookahead=True)
        run(gen_a(0))
        run(gen_a(1))
        load_wo(0)
        load_wo(1)
        interleave((gen_b(0), 1), (gen_a(2), 2))
        interleave((gen_c(0), 1), (chain(gen_a(3), take(b1, 9)), 1))
        interleave((b1, 1), (gen_a(4), 1))
        interleave((gen_c(1), 1), (chain(gen_a(5), take(b2, 9)), 1))
        interleave((b2, 3), (gen_a(6), 2))
        interleave((gen_c(2, 0, 4), 1), (chain(gen_a(7), take(b3, 9)), 2))
        interleave((b3, 4), (gen_c(2, 4, 8, tail=True), 1))
        run(gen_c(3, tail=True))

    return nc


def _make_in_maps(x, freqs_cis, wqkv, wo):
    scale = np.float32(1.0 / np.sqrt(HD))
    xT = np.ascontiguousarray(np.asarray(x)[0].T).astype(NPBF16)
    NCH, CHUNK = 4, 512
    cos = freqs_cis[:, :, 0].T.astype(np.float32)        # (64, S)
    sin = freqs_cis[:, :, 1].T.astype(np.float32)
    cosT = np.concatenate([cos, cos], 0)                 # (128, S)
    sinT = np.concatenate([-sin, sin], 0)
    # (128, NCH, 2, CHUNK): per chunk one contiguous cos||sin slab
    csT = np.ascontiguousarray(
        np.stack([cosT.reshape(128, NCH, CHUNK),
                  sinT.reshape(128, NCH, CHUNK)], axis=2))
    ones = np.ones((128, 128), NPBF16)
    kp = np.arange(128)[:, None]
    qp = np.arange(128)[None, :]
    maskT = np.where(kp <= qp, 0.0, -1e30).astype(np.float32)

    in_maps = []
    for c in range(N_CORES):
        rows = [wqkv[128 * (NHL * c + h) + PERM] * scale for h in range(NHL)]
        rows.append(wqkv[NH * HD + 128 * c + PERM])
        wqkT = np.concatenate(rows, 0).T                  # (DIM, 640)
        wvT = wqkv[(NH + NKV) * HD + 128 * c:(NH + NKV) * HD + 128 * (c + 1)].T
        wqkvT = np.ascontiguousarray(
            np.concatenate([wqkT, wvT], 1)).astype(NPBF16)  # (DIM, 768)
        woT = np.ascontiguousarray(
            wo[:, 128 * NHL * c:128 * NHL * (c + 1)].T).astype(NPBF16)
        in_maps.append({
            "xT": xT, "wqkvT": wqkvT, "woT": woT,
            "csT": csT, "onesW": ones, "maskT": maskT,
        })
    return in_maps


def kernel(x, freqs_cis, wqkv, wo):
    x = np.asarray(x, dtype=np.float32)
    freqs_cis = np.asarray(freqs_cis, dtype=np.float32)
    wqkv = np.asarray(wqkv, dtype=np.float32)
    wo = np.asarray(wo, dtype=np.float32)

    in_maps = _make_in_maps(x, freqs_cis, wqkv, wo)
    nc = bacc.Bacc("TRN2", target_bir_lowering=False, debug=False,
                   num_devices=N_CORES)
    build_attention_kernel(nc, S=S, DIM=DIM)
    nc.compile()
    res = run_bass_kernel_spmd(nc, in_maps, core_ids=list(range(N_CORES)))

    acc = np.zeros((S, DIM), np.float32)
    for r in res.results:
        acc += np.asarray(r["out"]).astype(np.float32)
    return acc[None]


# revision 12
# speedup vs baseline: 1.0771x; 1.0393x over previous
"""Tensor-parallel GQA attention block on 8 TRN2 NeuronCores (Bass/Tile).

Problem: B=1, S=2048, DIM=4096, 32 q heads / 8 kv heads (GQA), head_dim=128,
RoPE, causal softmax, output projection.

Sharding (tensor parallel by head, per the hint): core c of 8 owns q heads
4c..4c+3 and kv head c (GQA groups stay with their q heads). wqkv rows and wo
columns are sharded by head; attention is fully local per core; each core
emits a partial (S, DIM) output (its heads through its wo column slice) and
the partials are summed on the host at unshard time (the "all-reduce after
wo" of the hint, done off-device since full I/O passes through the host
anyway).

Per-core device kernel -- all operands host-pre-transposed so every matmul has
its contraction dim on SBUF partitions; zero on-device transposes:
  qkT = wqkT.T @ xT              (head dims on partitions, seq free)
  v   = xT.T @ wvT               (seq on partitions, head dim free)
  RoPE on qT/kT in transposed layout: host permutes rows into re(0..63)/
    im(64..127); cos/sin arrive as stacked (128, S) tables [cos;cos] and
    [-sin;sin]; 1/sqrt(HD) is folded into wq on the host.
  per head, per 512-wide q chunk (causal: only k tiles <= chunk end):
    S.T[j] = kT_j.T @ qT_chunk   (k positions on partitions => softmax
                                  denominators via a ones-matmul; no P
                                  transpose anywhere)
    P.T[j] = exp(S.T[j] - 12)    (triangular mask added on diagonal tiles;
                                  N trimmed to the causal columns)
    sums  += ones128.T @ P.T[j]  (PSUM-accumulated, rows replicated)
    O.T   += matmul(lhsT=V_j, rhs=P.T[j])
    O.T_norm = O.T * reciprocal_approx(sums)  -> bf16
  out[t, d] = sum_h O.T_h[:, t].T @ woT_h[:, d]

Schedule: quantum-interleaved emission.  Each phase is a generator of PE
"quanta" (~0.6-0.9us of matmuls); a weighted-fair driver mixes them so that
during attention the exp-dependent ones/PV matmuls always have independent
projection/output quanta between them and their score matmul -- the ACT
engine's exp latency hides under PE work instead of stalling it.  Each B
phase's first j-steps are pre-started inside the previous C interleave
(shared generator) so their counting-semaphore thresholds exclude C's
final PSUM evictions.  Weights stream as 9 staggered group DMAs of a
combined wqkv tensor (SWDGE descriptor generation is ~0.7us per DMA, so
descriptor count, not just bytes, gates startup); wo and per-chunk cos/sin
slices load outside the startup window.  PSUM->SBUF evictions alternate
ACT/DVE and out-DMAs rotate across engine queues (4-way at the tail).

Compute in bf16 with f32 PSUM accumulation; rel l2 error vs the f32
reference is ~8e-3.
"""
import sys

sys.path.insert(0, "/opt/trn_rl_repo")

from contextlib import ExitStack

import numpy as np
import ml_dtypes

import concourse.bass as bass
import concourse.tile as tile
import concourse.mybir as mybir
from concourse import bacc
from concourse.bass_utils import run_bass_kernel_spmd

F32 = mybir.dt.float32
BF16 = mybir.dt.bfloat16
NPBF16 = ml_dtypes.bfloat16

NH, NKV, HD = 32, 8, 128
S, DIM = 2048, 4096
N_CORES = 8
NHL = NH // N_CORES          # q heads per core
PERM = np.concatenate([np.arange(0, 128, 2), np.arange(1, 128, 2)])
WGROUPS = [2, 2, 4, 4, 4, 4, 4, 4, 4]   # k-tiles per weight-group DMA


def build_attention_kernel(nc, S=2048, DIM=4096, C=12.0):
    NHL = 4          # local q heads
    HD = 128
    CHUNK = 512
    P = 128
    NKT = DIM // P         # k tiles over model dim
    NCH = S // CHUNK       # seq chunks
    QKM = NHL + 1          # m-tiles in qk GEMM (4 q heads + 1 k head)
    NDC = DIM // CHUNK     # output dim chunks
    WM = QKM * P + HD      # combined wqkv row width (640 qk + 128 v)

    # ---- DRAM I/O ----
    xT = nc.dram_tensor("xT", (DIM, S), BF16, kind="ExternalInput").ap()
    # combined qkv weights: columns 0..639 = wqkT (4 q heads + 1 k head),
    # 640..767 = wvT -- one DMA per k-tile group covers both.
    wqkvT = nc.dram_tensor("wqkvT", (DIM, WM), BF16, kind="ExternalInput").ap()
    woT = nc.dram_tensor("woT", (NHL * HD, DIM), BF16, kind="ExternalInput").ap()
    # csT[:, ch, 0, :] = cos columns of chunk ch ([cos;cos] stacked rows),
    # csT[:, ch, 1, :] = sin columns ([-sin;+sin]) -- one DMA per chunk.
    csT = nc.dram_tensor("csT", (128, NCH, 2, CHUNK), F32,
                         kind="ExternalInput").ap()
    onesW = nc.dram_tensor("onesW", (P, P), BF16, kind="ExternalInput").ap()
    maskT = nc.dram_tensor("maskT", (P, P), F32, kind="ExternalInput").ap()
    out = nc.dram_tensor("out", (S, DIM), BF16, kind="ExternalOutput").ap()

    with tile.TileContext(nc) as tc, ExitStack() as ctx:
        const = ctx.enter_context(tc.tile_pool(name="const", bufs=1))
        resid = ctx.enter_context(tc.tile_pool(name="resid", bufs=1))
        xpool = ctx.enter_context(tc.tile_pool(name="xp", bufs=8))
        ptpool = ctx.enter_context(tc.tile_pool(name="ptp", bufs=6))
        tmppool = ctx.enter_context(tc.tile_pool(name="tmp", bufs=4))
        obpool = ctx.enter_context(tc.tile_pool(name="obp", bufs=8))
        psum = ctx.enter_context(tc.tile_pool(name="psum", bufs=8, space="PSUM"))

        # ---- weights: 9 staggered group DMAs (small groups first so the
        # first matmul starts in ~2us, larger ones amortize the per-DMA
        # SWDGE descriptor cost while the wire streams). ----
        w_g = []
        k0 = 0
        for gi, gn in enumerate(WGROUPS):
            g = const.tile([P, gn, WM], BF16, tag=f"wg{gi}", name=f"wg{gi}")
            nc.gpsimd.dma_start(
                g[:], wqkvT[k0 * P:(k0 + gn) * P, :].rearrange(
                    "(kt p) m -> p kt m", p=P))
            w_g.append((g, k0))
            k0 += gn
        ktile = []
        for (g, k0), gn in zip(w_g, WGROUPS):
            for i in range(gn):
                ktile.append(g[:, i])
        wqk_sb = [ktile[k][:, 0:QKM * P] for k in range(NKT)]
        wv_sb = [ktile[k][:, QKM * P:WM] for k in range(NKT)]
        ones_sb = const.tile([P, P], BF16, tag="ones", name="ones")
        nc.gpsimd.dma_start(ones_sb[:], onesW[:])
        mask_sb = const.tile([P, P], F32, tag="mask", name="mask")
        nc.gpsimd.dma_start(mask_sb[:], maskT[:])
        negC = const.tile([P, 1], F32, tag="negC", name="negC")
        nc.any.memset(negC[:], -C)
        cs_sb = const.tile([P, NCH, 2, CHUNK], F32, tag="cs", name="cs")
        wo_sb = const.tile([P, NHL, DIM], BF16, tag="wo", name="wo")

        cs_loaded = [False] * NCH

        def load_cs(ch):
            """cos/sin columns for chunk ch -- emitted at the start of the
            chunk's first A half so the slice has a full phase to arrive."""
            if cs_loaded[ch]:
                return
            cs_loaded[ch] = True
            nc.gpsimd.dma_start(cs_sb[:, ch], csT[:, ch])

        load_cs(0)
        load_cs(1)

        def load_wo(half):
            sl = slice(half * (DIM // 2), (half + 1) * (DIM // 2))
            nc.gpsimd.dma_start(
                wo_sb[:, :, sl],
                woT[:, sl].rearrange("(h p) n -> p h n", p=P))

        # resident activations (per chunk tiles for fine-grained deps)
        q_sb = [[resid.tile([P, CHUNK], BF16, tag=f"q{h}_{ch}", name=f"q{h}_{ch}")
                 for ch in range(NCH)] for h in range(NHL)]
        k_sb = [resid.tile([P, CHUNK], BF16, tag=f"k{ch}", name=f"k{ch}")
                for ch in range(NCH)]
        v_sb = [resid.tile([P, CHUNK], BF16, tag=f"v{ch}", name=f"v{ch}")
                for ch in range(NCH)]
        ot_sb = [[resid.tile([P, CHUNK], BF16, tag=f"ot{h}_{ch}", name=f"ot{h}_{ch}")
                  for ch in range(NCH)] for h in range(NHL)]

        def rope_hc(ps, raw_sw, out_tile, hc):
            """ps: (128, CHUNK//2) f32 PSUM [re; im]; raw_sw: bf16 SBUF with
            halves swapped [im; re] (produced by two ACT copies).
            out = ps*cosX + raw_sw*sinX with cosX = [cos; cos],
            sinX = [-sin; +sin]."""
            ch, half = hc // 2, hc % 2
            HC2 = CHUNK // 2
            cos = cs_sb[:, ch, 0, half * HC2:(half + 1) * HC2]
            sin = cs_sb[:, ch, 1, half * HC2:(half + 1) * HC2]
            t1 = tmppool.tile([P, HC2], F32, tag="t1", name="t1", bufs=3)
            t2 = tmppool.tile([P, HC2], F32, tag="t2", name="t2", bufs=3)
            nc.vector.tensor_mul(t1[:], ps[:], cos)
            nc.vector.tensor_mul(t2[:], raw_sw[:], sin)
            nc.vector.tensor_add(out_tile[:], t1[:], t2[:])

        HC = CHUNK // 2      # 256-wide half chunks: the qk PSUM footprint
        # is 3 banks (two heads packed per bank) + 1 shared V bank.
        vbank = [None]

        def gen_a(hc):
            """qkv projection + RoPE for seq half-chunk hc (generator: one
            quantum per k-tile, then the rope epilogue)."""
            ch, half = hc // 2, hc % 2
            load_cs(ch)
            qk_bank = [psum.tile([P, CHUNK], F32, tag="ps", name="ps")
                       for _ in range((QKM + 1) // 2)]
            if half == 0:
                vbank[0] = psum.tile([P, CHUNK], F32, tag="ps", name="ps")
            ps_v = vbank[0]

            def qk_slice(m):
                return qk_bank[m // 2][:, (m % 2) * HC:(m % 2 + 1) * HC]

            def fetch_x(k):
                xt = xpool.tile([P, HC], BF16, tag="xt", name="xt")
                # during the serial startup halves (hc 0,1) x alternates
                # between the sync and scalar HWDGE queues: one queue's
                # ~0.7us-per-DMA instruction rate is slower than the PE's
                # k-step consumption; ACT is idle there anyway.
                eng = nc.scalar if hc < 2 and k % 2 else nc.sync
                eng.dma_start(
                    xt[:], xT[k * P:(k + 1) * P, hc * HC:(hc + 1) * HC])
                return xt

            # deep explicit prefetch: with interleaved emission the k-step
            # quanta are spread out, so DMAs must be issued well ahead of
            # their consuming quantum.
            PF = 6
            xts = [fetch_x(k) for k in range(PF)]
            for k in range(NKT):
                xt = xts[k % PF]
                if k + PF < NKT:
                    xts[k % PF] = fetch_x(k + PF)
                for m in range(QKM):
                    nc.tensor.matmul(
                        qk_slice(m), wqk_sb[k][:, m * P:(m + 1) * P], xt[:],
                        start=(k == 0 and m % 2 == 0),
                        stop=(k == NKT - 1 and (m % 2 == 1 or m == QKM - 1)),
                        skip_group_check=True)
                for t in range(2):
                    nc.tensor.matmul(
                        ps_v[:, (2 * half + t) * P:(2 * half + t + 1) * P],
                        xt[:, t * P:(t + 1) * P], wv_sb[k][:],
                        start=(half == 0 and k == 0 and t == 0),
                        stop=(half == 1 and k == NKT - 1 and t == 1),
                        skip_group_check=True)
                yield
            if half == 1:
                nc.scalar.copy(v_sb[ch][:], ps_v[:])
            rawsw = [tmppool.tile([P, HC], BF16, tag=f"qksw{m}", name=f"qksw{m}", bufs=2)
                     for m in range(QKM)]
            order = [NHL] + list(range(NHL))     # k tile first
            for m in order:
                nc.scalar.copy(rawsw[m][0:64, :], qk_slice(m)[64:128, :])
                nc.scalar.copy(rawsw[m][64:128, :], qk_slice(m)[0:64, :])
            yield
            for m in order:
                out_tile = k_sb[ch] if m == NHL else q_sb[m][ch]
                rope_hc(qk_slice(m), rawsw[m],
                        out_tile[:, half * HC:(half + 1) * HC], hc)
                yield

        def gen_b(ch, lookahead=False):
            """attention for all local heads, q chunk ch (causal).  One
            quantum per (head, k-tile) step; the driver inserts an
            independent PE quantum in each gap so exp never stalls the PE.
            With lookahead=True the j+1 score matmul is also emitted before
            the exp-dependent sums/PV of j (used when little filler is
            available)."""
            njt = 4 * ch + 4

            def score(h, j):
                o = j - 4 * ch          # >=0: diagonal region, trim N
                lo = max(o, 0) * P      # first valid q column
                ps_st = psum.tile([P, CHUNK], F32, tag="ps", name="ps")
                nc.tensor.matmul(
                    ps_st[:, lo:], k_sb[j // 4][:, (j % 4) * P:(j % 4 + 1) * P],
                    q_sb[h][ch][:, lo:], start=True, stop=True,
                    skip_group_check=True)
                pt = ptpool.tile([P, CHUNK], BF16, tag="pt", name="pt")
                if o >= 0:  # mask the diagonal 128x128 block
                    nc.vector.tensor_add(
                        ps_st[:, o * P:(o + 1) * P],
                        ps_st[:, o * P:(o + 1) * P], mask_sb[:])
                nc.scalar.activation(
                    pt[:, lo:], ps_st[:, lo:],
                    mybir.ActivationFunctionType.Exp, bias=negC[:])
                return pt, lo

            for h in range(NHL):
                ps_sum = psum.tile([P, CHUNK], F32, tag="ps", name="ps")
                ps_ot = psum.tile([P, CHUNK], F32, tag="ps", name="ps")
                nxt = score(h, 0)
                for j in range(njt):
                    pt, lo = nxt
                    if lookahead and j + 1 < njt:
                        nxt = score(h, j + 1)
                    yield               # filler slot: exp(pt_j) runs here
                    if not lookahead and j + 1 < njt:
                        nxt = score(h, j + 1)
                    nc.tensor.matmul(ps_sum[:, lo:], ones_sb[:], pt[:, lo:],
                                     start=(j == 0), stop=(j == njt - 1),
                                     skip_group_check=True)
                    nc.tensor.matmul(
                        ps_ot[:, lo:], v_sb[j // 4][:, (j % 4) * P:(j % 4 + 1) * P],
                        pt[:, lo:], start=(j == 0), stop=(j == njt - 1),
                        skip_group_check=True)
                recip = tmppool.tile([P, CHUNK], F32, tag="recip", name="recip", bufs=2)
                nc.vector.reciprocal_approx_fast(out=recip[:], in_=ps_sum[:])
                nc.vector.tensor_mul(ot_sb[h][ch][:], ps_ot[:], recip[:])
                yield

        def gen_c(ch, dlo=0, dhi=None, tail=False):
            """output projection for the 4 seq tiles of chunk ch, output dim
            chunks dlo..dhi (generator: one quantum per (t, d) tile).
            PSUM->SBUF eviction alternates ACT/DVE; out-DMAs rotate across
            engine queues (4-way incl. sync when tail=True -- no x fetches
            follow that could be head-of-line blocked)."""
            if dhi is None:
                dhi = NDC
            qs = ([nc.sync, nc.gpsimd, nc.scalar] if tail
                  else [nc.gpsimd])
            qi = 0
            for tq in range(4):
                t = 4 * ch + tq
                for d in range(dlo, dhi):
                    ps_o = psum.tile([P, CHUNK], F32, tag="ps", name="ps")
                    for h in range(NHL):
                        nc.tensor.matmul(
                            ps_o[:], ot_sb[h][ch][:, tq * P:(tq + 1) * P],
                            wo_sb[:, h, d * CHUNK:(d + 1) * CHUNK],
                            start=(h == 0), stop=(h == NHL - 1),
                            skip_group_check=True)
                    ob = obpool.tile([P, CHUNK], BF16, tag="ob", name="ob")
                    if d % 2 == 0:
                        nc.vector.tensor_scalar_mul(ob[:], ps_o[:], 1.0)
                    else:
                        nc.scalar.copy(ob[:], ps_o[:])
                    qs[qi % len(qs)].dma_start(
                        out[t * P:(t + 1) * P, d * CHUNK:(d + 1) * CHUNK], ob[:])
                    qi += 1
                    yield

        def run(gen):
            for _ in gen:
                pass

        def take(gen, n):
            """yield up to n quanta from a shared generator."""
            for _ in range(n):
                if next(gen, "__done__") == "__done__":
                    return
                yield

        def chain(*gens):
            for g in gens:
                yield from g

        def interleave(*pairs):
            """pairs: (generator, weight).  Weighted fair queueing at
            single-quantum granularity: each step emits one quantum from the
            generator with the highest accumulated credit, so any ratio
            interleaves smoothly instead of in bursts."""
            state = [[g, float(w), 0.0] for g, w in pairs]
            while state:
                tot = sum(st[1] for st in state)
                for st in state:
                    st[2] += st[1] / tot
                st = max(state, key=lambda s: s[2])
                st[2] -= 1.0
                if next(st[0], "__done__") == "__done__":
                    state.remove(st)

        # ---- schedule ----
        # A(hc) covers chunk hc//2; B(b) needs chunks <= b roped and v'd;
        # C(b) needs all of B(b).  Fillers keep exp off the PE critical
        # path.  Each B's head is pre-started inside the previous C
        # interleave (after the A epilogue that ropes its chunk) so its
        # counting-semaphore thresholds exclude that C's final evictions.
        b1 = gen_b(1)
        b2 = gen_b(2)
        b3 = gen_b(3, lookahead=True)
        run(gen_a(0))
        run(gen_a(1))
        load_wo(0)
        load_wo(1)
        interleave((gen_b(0), 1), (gen_a(2), 2))
        interleave((gen_c(0), 2), (chain(gen_a(3), take(b1, 9)), 3))
        interleave((b1, 2), (gen_a(4), 3))
        interleave((gen_c(1), 2), (chain(gen_a(5), take(b2, 9)), 3))
        interleave((b2, 1), (gen_a(6), 1))
        interleave((gen_c(2, 0, 4), 1), (chain(gen_a(7), take(b3, 9)), 3))
        interleave((b3, 4), (gen_c(2, 4, 8, tail=True), 1))
        run(gen_c(3, tail=True))

    return nc


def _make_in_maps(x, freqs_cis, wqkv, wo):
    scale = np.float32(1.0 / np.sqrt(HD))
    xT = np.ascontiguousarray(np.asarray(x)[0].T).astype(NPBF16)
    NCH, CHUNK = 4, 512
    cos = freqs_cis[:, :, 0].T.astype(np.float32)        # (64, S)
    sin = freqs_cis[:, :, 1].T.astype(np.float32)
    cosT = np.concatenate([cos, cos], 0)                 # (128, S)
    sinT = np.concatenate([-sin, sin], 0)
    # (128, NCH, 2, CHUNK): per chunk one contiguous cos||sin slab
    csT = np.ascontiguousarray(
        np.stack([cosT.reshape(128, NCH, CHUNK),
                  sinT.reshape(128, NCH, CHUNK)], axis=2))
    ones = np.ones((128, 128), NPBF16)
    kp = np.arange(128)[:, None]
    qp = np.arange(128)[None, :]
    maskT = np.where(kp <= qp, 0.0, -1e30).astype(np.float32)

    in_maps = []
    for c in range(N_CORES):
        rows = [wqkv[128 * (NHL * c + h) + PERM] * scale for h in range(NHL)]
        rows.append(wqkv[NH * HD + 128 * c + PERM])
        wqkT = np.concatenate(rows, 0).T                  # (DIM, 640)
        wvT = wqkv[(NH + NKV) * HD + 128 * c:(NH + NKV) * HD + 128 * (c + 1)].T
        wqkvT = np.ascontiguousarray(
            np.concatenate([wqkT, wvT], 1)).astype(NPBF16)  # (DIM, 768)
        woT = np.ascontiguousarray(
            wo[:, 128 * NHL * c:128 * NHL * (c + 1)].T).astype(NPBF16)
        in_maps.append({
            "xT": xT, "wqkvT": wqkvT, "woT": woT,
            "csT": csT, "onesW": ones, "maskT": maskT,
        })
    return in_maps


def kernel(x, freqs_cis, wqkv, wo):
    x = np.asarray(x, dtype=np.float32)
    freqs_cis = np.asarray(freqs_cis, dtype=np.float32)
    wqkv = np.asarray(wqkv, dtype=np.float32)
    wo = np.asarray(wo, dtype=np.float32)

    in_maps = _make_in_maps(x, freqs_cis, wqkv, wo)
    nc = bacc.Bacc("TRN2", target_bir_lowering=False, debug=False,
                   num_devices=N_CORES)
    build_attention_kernel(nc, S=S, DIM=DIM)
    nc.compile()
    res = run_bass_kernel_spmd(nc, in_maps, core_ids=list(range(N_CORES)))

    acc = np.zeros((S, DIM), np.float32)
    for r in res.results:
        acc += np.asarray(r["out"]).astype(np.float32)
    return acc[None]
